# revision 1
# baseline (speedup 1.0000x reference)
"""DPLR SSM block kernel for Trainium2, 8 NeuronCores.

Math:  out = h @ (diag(a_diag) + p q^T).T + x @ b_mat          (B=64, H=8192, R=4)
           = h * a_diag  +  (h @ q) @ p^T  +  x @ b_mat

The dense (H,H) DPLR matrix is never materialized.  The memory-bound part is
streaming b_mat (256 MB fp32-worth of data).  Sharding: b_mat columns (= output
features) are split 8 ways; each core computes out[:, c*1024:(c+1)*1024].
x / q are replicated; host pre-permutes everything into the k-on-partitions
chunk layout the tensor engine wants, so no on-device transposes are needed.

fp32 matmul streams at 4 cycles/row on the PE, which would make the tensor
engine the critical path (~110us/core) over the DMA stream (~100us/core).
Instead x and b are carried as bf16 (hi, lo) pairs -- b ~= bh + bl with
bh = bf16(b), bl = bf16(b - bh) -- and the product uses three full-rate bf16
matmuls accumulating in fp32 PSUM:
    x @ b ~= xh@bh + xl@bh + xh@bl     (measured error ~4.6e-6, fp32-grade)
dropping only the xl@bl term (~2^-18 relative).  HBM traffic is unchanged
(2+2 bytes/element), but PE time drops to ~85us, restoring the DMA roofline.

Measured on trn2 (8 cores, looped-NEFF slope timing): ~119 us/core;
TimelineSim cost model predicts 117.8 us.  Idealized DMA roofline for the
36.6 MB/core stream at 368 GB/s is ~100 us.

Per core c (j0 = c*1024):
  hqT (4, 64)       = sum_ko  q[ko]^T(4x128) . hT[ko](128x64)          [PE fp32]
  ps  (64, 1024)    = 3-pass split-bf16 x @ b_slice                    [PE bf16]
                    + hqT^T(64x4) . pT(4x1024)                         [PE fp32]
  out (64, 1024)    = h_slice * a_slice  +  ps                         [DVE]
"""

import ml_dtypes
import numpy as np

import concourse.bass as bass
import concourse.mybir as mybir
from concourse import bacc
from concourse.bass_utils import run_bass_kernel_spmd
from concourse.tile import TileContext

H = 8192
R = 4
B = 64
NCORES = 8
JS = H // NCORES  # 1024 output columns per core
P = 128
KO = H // P  # 64 k-chunks
KT = 4  # k-chunks per DMA tile (tile = 128 x 4 x 2 x 1024 bf16 = 2 MB)
NT = KO // KT  # 16 b-mat DMA tiles per core

F32 = mybir.dt.float32
BF16 = mybir.dt.bfloat16
BF = ml_dtypes.bfloat16


def _build_nc(
    tiles: list[int] | None = None,
    bufs: int = 6,
    hq_tiles: tuple[int, int] = (4, 8),
    rank4_tile: int = 9,
    loop_n: int | None = None,
    aux_in_loop: bool = False,
    psum_split4: bool = False,
) -> bass.Bass:
    nc = bacc.Bacc("TRN2", target_bir_lowering=False, debug=False, num_devices=NCORES)

    xh = nc.dram_tensor("xh", (P, KO, B), BF16, kind="ExternalInput")
    xl = nc.dram_tensor("xl", (P, KO, B), BF16, kind="ExternalInput")
    ht = nc.dram_tensor("ht", (P, KO, B), F32, kind="ExternalInput")
    qk = nc.dram_tensor("qk", (P, KO, R), F32, kind="ExternalInput")
    pt = nc.dram_tensor("pt", (R, JS), F32, kind="ExternalInput")
    bm = nc.dram_tensor("bm", (P, KO, 2, JS), BF16, kind="ExternalInput")
    hs = nc.dram_tensor("hs", (B, JS), F32, kind="ExternalInput")
    ab = nc.dram_tensor("ab", (1, JS), F32, kind="ExternalInput")
    o = nc.dram_tensor("o", (B, JS), F32, kind="ExternalOutput")

    # b-tile sizes in k-chunks.  Tapered at both ends: small first tiles so
    # the PE can start as soon as possible, small last tiles so that after
    # the final DMA byte lands only one small tile's matmuls remain.
    TILES = tiles if tiles is not None else [1, 1, 2] + [4] * 14 + [2, 1, 1]
    assert sum(TILES) == KO
    MAXKT = max(TILES)

    with TileContext(nc) as tc:
        with (
            tc.tile_pool(name="persist", bufs=1) as persist,
            tc.tile_pool(name="bpool", bufs=bufs) as bpool,
            tc.tile_pool(name="psum", bufs=1, space="PSUM") as psum_pool,
        ):
            # Aux loads on the scalar HWDGE ring so the b stream on nc.sync
            # isn't delayed.  qk/ht-chunks/xh/xl are ordered so the PE's hq
            # matmul groups and first main tiles can start as early as
            # possible; hq groups are interleaved between the first main
            # tiles to fill the PE while the DMA stream warms up.
            xh_sb = persist.tile([P, KO, B], BF16)
            xl_sb = persist.tile([P, KO, B], BF16)
            qk_sb = persist.tile([P, KO, R], F32)
            ht_sb = persist.tile([P, KO, B], F32)
            pt_sb = persist.tile([R, JS], F32)
            hs_sb = persist.tile([B, JS], F32)
            a1_sb = persist.tile([1, JS], F32)
            ab_sb = persist.tile([B, JS], F32)

            def emit_aux():
                nc.scalar.dma_start(out=xh_sb[:], in_=xh[:, :, :])
                nc.scalar.dma_start(out=xl_sb[:], in_=xl[:, :, :])
                nc.scalar.dma_start(out=qk_sb[:], in_=qk[:, :, :])
                HT_CH = KO // 4
                for hc in range(4):
                    ksl = slice(hc * HT_CH, (hc + 1) * HT_CH)
                    nc.scalar.dma_start(out=ht_sb[:, ksl], in_=ht[:, ksl, :])
                nc.scalar.dma_start(out=pt_sb[:], in_=pt[:, :])
                nc.scalar.dma_start(out=hs_sb[:], in_=hs[:, :])
                # a_diag slice arrives as one row; broadcast to all 64 batch
                # partitions on the (otherwise idle) GPSIMD engine.
                nc.scalar.dma_start(out=a1_sb[:], in_=ab[:, :])
                nc.gpsimd.partition_broadcast(ab_sb[:], a1_sb[:])

            out_sb = persist.tile([B, JS], F32)
            hqt_sb = persist.tile([R, B], F32)

            import contextlib

            loop_ctx = (
                tc.For_i(0, loop_n, 1, hint_engines=(mybir.EngineType.PE,))
                if loop_n
                else contextlib.nullcontext()
            )
            if not (loop_n and aux_in_loop):
                emit_aux()
            with loop_ctx:
                if loop_n and aux_in_loop:
                    emit_aux()
                _emit_body(
                    nc, tc, TILES, MAXKT, bpool, psum_pool, persist,
                    qk_sb, ht_sb, xh_sb, xl_sb, pt_sb, hs_sb, ab_sb,
                    out_sb, hqt_sb, bm, o, hq_tiles, rank4_tile, psum_split4,
                )

    nc.finalize()
    return nc


def _emit_body(
    nc, tc, TILES, MAXKT, bpool, psum_pool, persist,
    qk_sb, ht_sb, xh_sb, xl_sb, pt_sb, hs_sb, ab_sb,
    out_sb, hqt_sb, bm, o, hq_tiles, rank4_tile, psum_split4=False,
):
            ps0 = psum_pool.tile([B, 512], F32)
            ps1 = psum_pool.tile([B, 512], F32)
            ps2 = psum_pool.tile([B, 512], F32)
            ps3 = psum_pool.tile([B, 512], F32)
            pshq = psum_pool.tile([R, B], F32)

            # Diagonal term early (off the critical tail).
            nc.vector.tensor_mul(out=out_sb[:], in0=hs_sb[:], in1=ab_sb[:])

            hq_done = [0]

            def hq_emit(n):
                # hqT = q^T @ h^T: emit the next n k-chunks (fp32).
                for ko in range(hq_done[0], min(hq_done[0] + n, KO)):
                    nc.tensor.matmul(
                        pshq[:],
                        qk_sb[:, ko],
                        ht_sb[:, ko],
                        start=(ko == 0),
                        stop=(ko == KO - 1),
                    )
                hq_done[0] = min(hq_done[0] + n, KO)

            def hq_group(g):
                hq_emit(16)

            # Main stream: x @ b_slice via 3-pass split-bf16.
            ko = 0
            for t, kt in enumerate(TILES):
                if hq_tiles[0] <= t < hq_tiles[1]:
                    ng = hq_tiles[1] - hq_tiles[0]
                    # Spread the 64 hq matmuls evenly over the window so
                    # they fill the PE's per-tile DMA-wait bubbles.
                    per = (KO + ng - 1) // ng
                    hq_emit(per)
                if t == rank4_tile:
                    hq_emit(KO)  # any remainder before the rank-4 term
                    # Rank-4 term into its own PSUM banks, mid-stream.
                    nc.vector.tensor_copy(out=hqt_sb[:], in_=pshq[:])
                    nc.tensor.matmul(
                        ps2[:], hqt_sb[:], pt_sb[:, 0:512], start=True, stop=True
                    )
                    nc.tensor.matmul(
                        ps3[:], hqt_sb[:], pt_sb[:, 512:JS], start=True, stop=True
                    )
                bfull = bpool.tile([P, MAXKT, 2, JS], BF16, name="btile")
                btile = bfull[:, :kt]
                dma_eng = nc.sync if t % 2 == 0 else nc.scalar
                dma_eng.dma_start(out=btile[:], in_=bm[:, ko : ko + kt])
                for k4 in range(kt):
                    st = ko == 0
                    lst = ko == KO - 1
                    bh = btile[:, k4, 0]
                    bl = btile[:, k4, 1]
                    if psum_split4:
                        # 4x N=256 matmuls per pass: marginally finer
                        # PE/DMA lockstep granularity (sim: -315 ns).
                        for qi, pq in enumerate((ps0, ps1)):
                            for hf in (0, 1):
                                sl = slice((2 * qi + hf) * 256, (2 * qi + hf + 1) * 256)
                                po = pq[:, hf * 256 : (hf + 1) * 256]
                                nc.tensor.matmul(
                                    po, xh_sb[:, ko], bh[:, sl], start=st, stop=False
                                )
                                nc.tensor.matmul(
                                    po, xh_sb[:, ko], bl[:, sl], start=False, stop=False
                                )
                                nc.tensor.matmul(
                                    po, xl_sb[:, ko], bh[:, sl], start=False, stop=lst
                                )
                        ko += 1
                        continue
                    nc.tensor.matmul(
                        ps0[:], xh_sb[:, ko], bh[:, 0:512], start=st, stop=False
                    )
                    nc.tensor.matmul(
                        ps1[:], xh_sb[:, ko], bh[:, 512:JS], start=st, stop=False
                    )
                    nc.tensor.matmul(
                        ps0[:], xh_sb[:, ko], bl[:, 0:512], start=False, stop=False
                    )
                    nc.tensor.matmul(
                        ps1[:], xh_sb[:, ko], bl[:, 512:JS], start=False, stop=False
                    )
                    # xl-stationary last: xl arrives after xh at startup.
                    nc.tensor.matmul(
                        ps0[:], xl_sb[:, ko], bh[:, 0:512], start=False, stop=lst
                    )
                    nc.tensor.matmul(
                        ps1[:], xl_sb[:, ko], bh[:, 512:JS], start=False, stop=lst
                    )
                    ko += 1

            # Rank-4 folded into out_sb mid-stream (off the critical tail).
            nc.vector.tensor_add(
                out=out_sb[:, 0:512], in0=out_sb[:, 0:512], in1=ps2[:]
            )
            nc.vector.tensor_add(
                out=out_sb[:, 512:JS], in0=out_sb[:, 512:JS], in1=ps3[:]
            )

            # Tail: fold the main accumulators and store.
            nc.vector.tensor_add(
                out=out_sb[:, 0:512], in0=out_sb[:, 0:512], in1=ps0[:]
            )
            nc.sync.dma_start(out=o[:, 0:512], in_=out_sb[:, 0:512])
            nc.vector.tensor_add(
                out=out_sb[:, 512:JS], in0=out_sb[:, 512:JS], in1=ps1[:]
            )
            nc.scalar.dma_start(out=o[:, 512:JS], in_=out_sb[:, 512:JS])


_NC_CACHE = None


def _get_nc() -> bass.Bass:
    global _NC_CACHE
    if _NC_CACHE is None:
        _NC_CACHE = _build_nc()
    return _NC_CACHE


def _split_bf16(a: np.ndarray) -> tuple[np.ndarray, np.ndarray]:
    hi = a.astype(BF)
    lo = (a - hi.astype(np.float32)).astype(BF)
    return hi, lo


def _in_maps(h, x, a_diag, p_vec, q_vec, b_mat):
    # Replicated inputs, pre-permuted to k-on-partitions chunk layout.
    # xt[ki, ko, b] = x[b, ko*128 + ki]
    xt = np.ascontiguousarray(x.reshape(B, KO, P).transpose(2, 1, 0))
    xh, xl = _split_bf16(xt)
    ht = np.ascontiguousarray(h.reshape(B, KO, P).transpose(2, 1, 0))
    # qk[ki, ko, r] = q_vec[ko*128 + ki, r]
    qk = np.ascontiguousarray(q_vec.reshape(KO, P, R).transpose(1, 0, 2))

    # b4[ko, ki, c, j] = b_mat[ko*128 + ki, c*1024 + j]
    b4 = b_mat.reshape(KO, P, NCORES, JS)

    in_maps = []
    for c in range(NCORES):
        j0 = c * JS
        bc = np.ascontiguousarray(b4[:, :, c, :].transpose(1, 0, 2))  # (P, KO, JS)
        bh, bl = _split_bf16(bc)
        bhl = np.ascontiguousarray(np.stack([bh, bl], axis=2))  # (P, KO, 2, JS)
        in_maps.append(
            {
                "xh": xh,
                "xl": xl,
                "ht": ht,
                "qk": qk,
                "pt": np.ascontiguousarray(p_vec[j0 : j0 + JS, :].T),
                "bm": bhl,
                "hs": np.ascontiguousarray(h[:, j0 : j0 + JS]),
                "ab": np.ascontiguousarray(a_diag[j0 : j0 + JS]).reshape(1, JS),
            }
        )
    return in_maps


def kernel(h, x, a_diag, p_vec, q_vec, b_mat) -> np.ndarray:
    h = np.ascontiguousarray(np.asarray(h, dtype=np.float32))
    x = np.ascontiguousarray(np.asarray(x, dtype=np.float32))
    a_diag = np.asarray(a_diag, dtype=np.float32)
    p_vec = np.asarray(p_vec, dtype=np.float32)
    q_vec = np.asarray(q_vec, dtype=np.float32)
    b_mat = np.asarray(b_mat, dtype=np.float32)

    nc = _get_nc()
    res = run_bass_kernel_spmd(
        nc, _in_maps(h, x, a_diag, p_vec, q_vec, b_mat), core_ids=list(range(NCORES))
    )
    return np.concatenate([r["o"] for r in res.results], axis=1)



# revision 2
# speedup vs baseline: 2.9825x; 2.9825x over previous
"""DPLR SSM block kernel for Trainium2, 8 NeuronCores.

Math:  out = h @ (diag(a_diag) + p q^T).T + x @ b_mat          (B=64, H=8192, R=4)
           = h * a_diag  +  (h @ q) @ p^T  +  x @ b_mat

The dense (H,H) DPLR matrix is never materialized.  Sharding: b_mat columns
(= output features) split 8 ways; core c computes out[:, c*1024:(c+1)*1024].

The problem is HBM-bound on streaming b_mat.  The correctness budget
(rel_err < 2e-2) is spent on an aggressive quantization of b: fp8 e3m4
(1 byte/elem) with a global scale S chosen so max|S*b| lands just under the
e3m4 max-finite (15.5).  The descale 1/S is folded into the x operand on the
host (x/S carried as fp16), so the device does no rescaling at all.
Measured end-to-end rel_fro error: 1.21e-2 (gate 2e-2).

Per-core HBM traffic drops from 36.6 MB (baseline split-bf16) to ~10.4 MB:
b 8 MB fp8 + x 1 MB fp16 + h 1 MB fp16 + ~0.4 MB small aux + 0.25 MB out.

PE layout puts the 64-wide batch on the MOVING operand (x chunks) and b
column-chunks (128 wide) on the stationary side, so each 128k-chunk costs
8 matmuls x 64 moving rows = 512 rows instead of 1024 -- ~16 us of PE for a
~29 us DMA stream.  The rank-4 term is accumulated by the PE directly into
the same PSUM bank mid-stream; the diagonal term rides in the single fused
DVE epilogue op per column group:
    out[:,g,:] = (hd_g * a_g) + psum_g        (scalar_tensor_tensor)

Output is produced transposed ([feature, batch]) and untransposed on host.

Per core c (j0 = c*1024, groups g of 128 columns):
  psA[:,g,:] (128,64) = sum_ko  bS[ko,g]^T(128x128) . xq[ko](128x64)   [PE, fp8xfp16]
                      + ptS[g]^T(4x128-slice) . hqT(4x64)              [PE, fp16]
  pshq (4,64)         = sum_ko  qk[ko]^T(128x4) . ht[ko](128x64)       [PE, fp16]
  out[:,g,:]          = hd[:,g,:] * ad[:,g]  +  psA[:,g,:]             [DVE]
"""

import ml_dtypes
import numpy as np

import concourse.bass as bass
import concourse.mybir as mybir
from concourse import bacc
from concourse.bass_utils import run_bass_kernel_spmd
from concourse.tile import TileContext

H = 8192
R = 4
B = 64
NCORES = 8
JS = H // NCORES  # 1024 output columns per core
P = 128
G = JS // P  # 8 column groups of 128 per core
KO = H // P  # 64 k-chunks

F32 = mybir.dt.float32
F16 = mybir.dt.float16
F8 = mybir.dt.float8e3

NP_F16 = np.float16
NP_F8 = ml_dtypes.float8_e3m4
E3M4_MAX_SAFE = 15.49  # just under e3m4 max finite (15.5); no overflow to inf


def _build_nc(
    tiles: list[int] | None = None,
    bufs: int = 6,
    hq_start: int = 3,
    hq_per_tile: int = 8,
    rank4_tile: int = 12,
) -> bass.Bass:
    nc = bacc.Bacc("TRN2", target_bir_lowering=False, debug=False, num_devices=NCORES)

    xq = nc.dram_tensor("xq", (P, KO, B), F16, kind="ExternalInput")
    ht = nc.dram_tensor("ht", (P, KO, B), F16, kind="ExternalInput")
    qk = nc.dram_tensor("qk", (P, KO, R), F16, kind="ExternalInput")
    pt = nc.dram_tensor("pt", (R, JS), F16, kind="ExternalInput")
    bm = nc.dram_tensor("bm", (P, KO, G, P), F8, kind="ExternalInput")
    hd = nc.dram_tensor("hd", (P, G, B), F16, kind="ExternalInput")
    ad = nc.dram_tensor("ad", (P, G), F32, kind="ExternalInput")
    o = nc.dram_tensor("o", (P, G, B), F32, kind="ExternalOutput")

    # b-tile sizes in k-chunks.  Small tail tiles so that once the final DMA
    # byte lands only one chunk's worth of matmuls + the epilogue remain.
    TILES = tiles if tiles is not None else [4] * 15 + [2, 1, 1]
    assert sum(TILES) == KO
    MAXKT = max(TILES)

    with TileContext(nc) as tc:
        with (
            tc.tile_pool(name="persist", bufs=1) as persist,
            tc.tile_pool(name="bpool", bufs=bufs) as bpool,
            tc.tile_pool(name="psum", bufs=1, space="PSUM") as psum_pool,
        ):
            xq_sb = persist.tile([P, KO, B], F16)
            ht_sb = persist.tile([P, KO, B], F16)
            qk_sb = persist.tile([P, KO, R], F16)
            pt_sb = persist.tile([R, JS], F16)
            hd_sb = persist.tile([P, G, B], F16)
            ad_sb = persist.tile([P, G], F32)
            out_sb = persist.tile([P, G, B], F32)
            hqt_sb = persist.tile([R, B], F16)

            psA = psum_pool.tile([P, G, B], F32)  # exactly one PSUM bank
            pshq = psum_pool.tile([R, B], F32)

            # Leading aux: the x chunks the PE needs first, then the rest.
            # Everything rides the scalar-engine HWDGE ring; b tiles alternate
            # sync/scalar.  Transfers serialize on the DMA engines globally,
            # so order of emission ~= order on the wire.
            nc.scalar.dma_start(out=xq_sb[:, 0:8], in_=xq[:, 0:8])
            nc.scalar.dma_start(out=qk_sb[:], in_=qk[:, :, :])
            nc.scalar.dma_start(out=xq_sb[:, 8:KO], in_=xq[:, 8:KO])

            # ht pieces + small aux are interleaved between early b tiles via
            # emit_aux(t) below: hq consumes ht piece i from tile 3+2i on.
            HT_CH = KO // 4

            def emit_aux(t):
                if t == 1:
                    nc.scalar.dma_start(
                        out=ht_sb[:, 0:HT_CH], in_=ht[:, 0:HT_CH]
                    )
                elif t == 3:
                    nc.scalar.dma_start(
                        out=ht_sb[:, HT_CH : 2 * HT_CH], in_=ht[:, HT_CH : 2 * HT_CH]
                    )
                elif t == 5:
                    nc.scalar.dma_start(
                        out=ht_sb[:, 2 * HT_CH : 3 * HT_CH],
                        in_=ht[:, 2 * HT_CH : 3 * HT_CH],
                    )
                elif t == 7:
                    nc.scalar.dma_start(
                        out=ht_sb[:, 3 * HT_CH : KO], in_=ht[:, 3 * HT_CH : KO]
                    )
                elif t == 9:
                    nc.scalar.dma_start(out=hd_sb[:], in_=hd[:, :, :])
                    nc.scalar.dma_start(out=ad_sb[:], in_=ad[:, :])
                    nc.scalar.dma_start(out=pt_sb[:], in_=pt[:, :])

            hq_done = [0]

            def hq_emit(n):
                # pshq = q^T @ h^T: emit the next n k-chunks (fp16).
                for ko in range(hq_done[0], min(hq_done[0] + n, KO)):
                    nc.tensor.matmul(
                        pshq[:],
                        qk_sb[:, ko],
                        ht_sb[:, ko],
                        start=(ko == 0),
                        stop=(ko == KO - 1),
                    )
                hq_done[0] = min(hq_done[0] + n, KO)

            # Main stream.  All G groups of one PSUM bank share a single
            # start (first matmul zeroes the whole 2KB zero region) and a
            # single stop (very last matmul into the bank).
            ko = 0
            n_main = KO * G
            im = 0
            for t, kt in enumerate(TILES):
                bfull = bpool.tile([P, MAXKT, G, P], F8, name="btile")
                btile = bfull[:, :kt]
                dma_eng = nc.sync if t % 2 == 0 else nc.scalar
                dma_eng.dma_start(out=btile[:], in_=bm[:, ko : ko + kt])
                emit_aux(t)
                if t >= hq_start:
                    hq_emit(hq_per_tile)
                if t == rank4_tile:
                    hq_emit(KO)  # any remainder before the rank-4 term
                    nc.vector.tensor_copy(out=hqt_sb[:], in_=pshq[:])
                    for g in range(G):
                        nc.tensor.matmul(
                            psA[:, g],
                            pt_sb[:, g * P : (g + 1) * P],
                            hqt_sb[:],
                            start=False,
                            stop=False,
                        )
                for k4 in range(kt):
                    for g in range(G):
                        nc.tensor.matmul(
                            psA[:, g],
                            btile[:, k4, g],
                            xq_sb[:, ko],
                            start=(im == 0),
                            stop=(im == n_main - 1),
                        )
                        im += 1
                    ko += 1

            # Epilogue: one fused DVE op per group folds the diagonal term
            # and moves PSUM->SBUF; two DMAs overlap the store tail.
            for g in range(G):
                nc.vector.scalar_tensor_tensor(
                    out=out_sb[:, g],
                    in0=hd_sb[:, g],
                    scalar=ad_sb[:, g : g + 1],
                    in1=psA[:, g],
                    op0=mybir.AluOpType.mult,
                    op1=mybir.AluOpType.add,
                )
                if g == G // 2 - 1:
                    nc.sync.dma_start(
                        out=o[:, 0 : G // 2], in_=out_sb[:, 0 : G // 2]
                    )
            nc.scalar.dma_start(out=o[:, G // 2 : G], in_=out_sb[:, G // 2 : G])

    nc.finalize()
    return nc


_NC_CACHE = None


def _get_nc() -> bass.Bass:
    global _NC_CACHE
    if _NC_CACHE is None:
        _NC_CACHE = _build_nc()
    return _NC_CACHE


def _in_maps(h, x, a_diag, p_vec, q_vec, b_mat):
    bmax = float(np.abs(b_mat).max())
    S = E3M4_MAX_SAFE / bmax if bmax > 0 else 1.0

    # Replicated inputs, pre-permuted to k-on-partitions chunk layout.
    # xq[ki, ko, b] = x[b, ko*128 + ki] / S   (descale folded into x)
    xq = np.ascontiguousarray(
        (x / S).astype(NP_F16).reshape(B, KO, P).transpose(2, 1, 0)
    )
    ht = np.ascontiguousarray(h.astype(NP_F16).reshape(B, KO, P).transpose(2, 1, 0))
    qk = np.ascontiguousarray(
        q_vec.astype(NP_F16).reshape(KO, P, R).transpose(1, 0, 2)
    )

    # Whole-b quantization once: bq[ko, ki, c, g, j] = S*b[ko*128+ki, ...]
    bq = (b_mat * S).astype(NP_F8).reshape(KO, P, NCORES, G, P)

    in_maps = []
    for c in range(NCORES):
        j0 = c * JS
        bc = np.ascontiguousarray(bq[:, :, c].transpose(1, 0, 2, 3))  # (P,KO,G,P)
        # hd[j, g, b] = h[b, j0 + g*128 + j];  ad[j, g] = a_diag[j0 + g*128 + j]
        hslice = h[:, j0 : j0 + JS].astype(NP_F16)  # (B, JS)
        hd = np.ascontiguousarray(hslice.reshape(B, G, P).transpose(2, 1, 0))
        adc = np.ascontiguousarray(
            a_diag[j0 : j0 + JS].reshape(G, P).T.astype(np.float32)
        )
        in_maps.append(
            {
                "xq": xq,
                "ht": ht,
                "qk": qk,
                "pt": np.ascontiguousarray(p_vec[j0 : j0 + JS, :].T.astype(NP_F16)),
                "bm": bc,
                "hd": hd,
                "ad": adc,
            }
        )
    return in_maps


def kernel(h, x, a_diag, p_vec, q_vec, b_mat) -> np.ndarray:
    h = np.ascontiguousarray(np.asarray(h, dtype=np.float32))
    x = np.ascontiguousarray(np.asarray(x, dtype=np.float32))
    a_diag = np.asarray(a_diag, dtype=np.float32)
    p_vec = np.asarray(p_vec, dtype=np.float32)
    q_vec = np.asarray(q_vec, dtype=np.float32)
    b_mat = np.asarray(b_mat, dtype=np.float32)

    nc = _get_nc()
    res = run_bass_kernel_spmd(
        nc, _in_maps(h, x, a_diag, p_vec, q_vec, b_mat), core_ids=list(range(NCORES))
    )
    # o[j, g, b] -> out[:, c*1024 + g*128 + j]
    outs = [
        np.asarray(r["o"]).transpose(2, 1, 0).reshape(B, JS) for r in res.results
    ]
    return np.concatenate(outs, axis=1).astype(np.float32)


# revision 4
# speedup vs baseline: 3.1510x; 1.0565x over previous
"""DPLR SSM block kernel for Trainium2, 8 NeuronCores.

Math:  out = h @ (diag(a_diag) + p q^T).T + x @ b_mat          (B=64, H=8192, R=4)
           = h * a_diag  +  (h @ q) @ p^T  +  x @ b_mat

The dense (H,H) DPLR matrix is never materialized.  Sharding: b_mat columns
(= output features) split 8 ways; core c computes out[:, c*1024:(c+1)*1024].

HBM-bound on streaming b_mat; the correctness budget (rel_err < 2e-2) is
spent on quantization:
  - b as fp8 e3m4 (1 byte/elem), global scale S with max|S*b| ~ 15.49 (just
    under e3m4 max finite).  The descale is folded into x on the host
    (xq = x/S as fp16), so the device never rescales.
  - h/q feeding the tiny rank-4 term ride as e3m4 too (2h and 512q; the
    1/1024 descale folds into the one hq PSUM->SBUF copy).
  - output stored as fp16, upcast on host.
Measured end-to-end rel_fro error: ~1.21e-2 (gate 2e-2).

Per-core HBM traffic: b 8 MB + x 1 MB fp16 + h 0.5 MB fp8 + ~0.2 MB small
aux + 0.125 MB out ~ 9.7 MB (~27 us at the 360 GB/s modeled DMA roofline),
vs 36.6 MB for the baseline split-bf16 kernel.

PE layout: batch (64) on the moving operand, b column-blocks (128 wide)
stationary -> 64 moving rows per (chunk, group) matmul: ~16 us PE, under the
DMA roofline.  b streams GROUP-MAJOR (all 64 k-chunks of a 128-column group
consecutively), so group g's PSUM accumulation finishes at (g+1)/8 of the
stream and its epilogue + 16 KB output DMA ride mid-stream; only the last
group's epilogue is on the tail.  The rank-4 term goes through a second PSUM
bank and is pre-folded with the diagonal term into out_sb mid-stream:
    fold (mid-stream):  out_sb[:,g,:] = hd_g * ad_g + psR_g     [DVE, fused]
    epilogue per group: out_sb[:,g,:] += psA_g ; DMA out        [DVE ring]

Per core c (j0 = c*1024, groups g of 128 columns):
  psA[:,g,:] (128,64) = sum_ko  bS[g,ko]^T(128x128) . xq[ko](128x64) [PE f8xf16]
  pshq (4,64)         = sum_ko  qk[ko]^T(128x4) . ht[ko](128x64)     [PE f8]
  hqt (4,64)          = pshq / 1024                                  [DVE]
  psR[:,g,:] (128,64) = pt[g]^T(4x128) . hqt(4x64)                   [PE f16]
"""

import ml_dtypes
import numpy as np

import concourse.bass as bass
import concourse.mybir as mybir
from concourse import bacc
from concourse.bass_utils import run_bass_kernel_spmd
from concourse.tile import TileContext

H = 8192
R = 4
B = 64
NCORES = 8
JS = H // NCORES  # 1024 output columns per core
P = 128
G = JS // P  # 8 column groups of 128 per core
KO = H // P  # 64 k-chunks

F32 = mybir.dt.float32
F16 = mybir.dt.float16
F8 = mybir.dt.float8e3

NP_F16 = np.float16
NP_F8 = ml_dtypes.float8_e3m4
E3M4_MAX_SAFE = 15.49  # just under e3m4 max finite (15.5); no overflow to inf
HT_SCALE = 2.0  # pre-scale on h for the hq matmul (fewer e3m4 subnormals)
QK_SCALE = 512.0  # pre-scale on q (e3m4 subnormal floor is 2^-6)

# b-tile taper in k-chunks per group: big tiles for full-rate streaming,
# small tail tiles on the LAST group so the final-byte -> final-matmul ->
# epilogue chain is short.
TILES_MID = [32, 32]
TILES_LAST = [32, 16, 8, 4, 2, 1, 1]
MAXKT = 32


def _build_nc(bufs: int = 6) -> bass.Bass:
    nc = bacc.Bacc("TRN2", target_bir_lowering=False, debug=False, num_devices=NCORES)

    xq = nc.dram_tensor("xq", (P, KO, B), F16, kind="ExternalInput")
    ht = nc.dram_tensor("ht", (P, KO, B), F8, kind="ExternalInput")
    qk = nc.dram_tensor("qk", (P, KO, R), F8, kind="ExternalInput")
    pt = nc.dram_tensor("pt", (R, JS), F16, kind="ExternalInput")
    bm = nc.dram_tensor("bm", (G, P, KO, P), F8, kind="ExternalInput")
    hd = nc.dram_tensor("hd", (P, G, B), F16, kind="ExternalInput")
    ad = nc.dram_tensor("ad", (P, G), F32, kind="ExternalInput")
    o = nc.dram_tensor("o", (P, G, B), F16, kind="ExternalOutput")

    with TileContext(nc) as tc:
        with (
            tc.tile_pool(name="persist", bufs=1) as persist,
            tc.tile_pool(name="bpool", bufs=bufs) as bpool,
            tc.tile_pool(name="psum", bufs=1, space="PSUM") as psum_pool,
        ):
            xq_sb = persist.tile([P, KO, B], F16)
            ht_sb = persist.tile([P, KO, B], F8)
            qk_sb = persist.tile([P, KO, R], F8)
            pt_sb = persist.tile([R, JS], F16)
            hd_sb = persist.tile([P, G, B], F16)
            ad_sb = persist.tile([P, G], F32)
            out_sb = persist.tile([P, G, B], F16)
            hqt_sb = persist.tile([R, B], F16)

            psA = psum_pool.tile([P, G, B], F32)  # main accum, one PSUM bank
            psR = psum_pool.tile([P, G, B], F32)  # rank-4 term, one PSUM bank
            pshq = psum_pool.tile([R, B], F32)

            # Aux stream on the Activation HWDGE ring.  xq first (the first
            # main matmuls need it), everything else behind it; b tiles ride
            # the SP ring concurrently and interleave at the DMA engines.
            nc.scalar.dma_start(out=xq_sb[:, 0:MAXKT], in_=xq[:, 0:MAXKT])
            nc.scalar.dma_start(out=xq_sb[:, MAXKT:KO], in_=xq[:, MAXKT:KO])
            nc.scalar.dma_start(out=ht_sb[:], in_=ht[:, :, :])
            nc.scalar.dma_start(out=qk_sb[:], in_=qk[:, :, :])
            nc.scalar.dma_start(out=hd_sb[:], in_=hd[:, :, :])
            nc.scalar.dma_start(out=ad_sb[:], in_=ad[:, :])
            nc.scalar.dma_start(out=pt_sb[:], in_=pt[:, :])

            n_main = KO * G
            im = 0
            for g in range(G):
                tiles = TILES_LAST if g == G - 1 else TILES_MID
                ko = 0
                for kt in tiles:
                    bfull = bpool.tile([P, MAXKT, P], F8, name="btile")
                    btile = bfull[:, :kt]
                    nc.sync.dma_start(out=btile[:], in_=bm[g, :, ko : ko + kt])
                    for k4 in range(kt):
                        nc.tensor.matmul(
                            psA[:, g],
                            btile[:, k4],
                            xq_sb[:, ko],
                            start=(im == 0),
                            stop=(im == n_main - 1),
                        )
                        im += 1
                        ko += 1

                if g == 1:
                    # After two groups of mains the PE has plenty of slack
                    # and ht/qk/pt have landed: fold the whole DPLR low-rank
                    # path here, mid-stream.
                    for ko_q in range(KO):
                        nc.tensor.matmul(
                            pshq[:],
                            qk_sb[:, ko_q],
                            ht_sb[:, ko_q],
                            start=(ko_q == 0),
                            stop=(ko_q == KO - 1),
                        )
                    nc.vector.tensor_scalar_mul(
                        hqt_sb[:], pshq[:], 1.0 / (HT_SCALE * QK_SCALE)
                    )
                    for gr in range(G):
                        nc.tensor.matmul(
                            psR[:, gr],
                            pt_sb[:, gr * P : (gr + 1) * P],
                            hqt_sb[:],
                            start=(gr == 0),
                            stop=(gr == G - 1),
                        )
                    for gr in range(G):
                        nc.vector.scalar_tensor_tensor(
                            out=out_sb[:, gr],
                            in0=hd_sb[:, gr],
                            scalar=ad_sb[:, gr : gr + 1],
                            in1=psR[:, gr],
                            op0=mybir.AluOpType.mult,
                            op1=mybir.AluOpType.add,
                        )
                    # groups 0 and 1 finished their mains before the fold:
                    # emit their epilogues now.
                    for gr in (0, 1):
                        nc.vector.tensor_add(
                            out=out_sb[:, gr], in0=out_sb[:, gr], in1=psA[:, gr]
                        )
                        nc.scalar.dma_start(out=o[:, gr], in_=out_sb[:, gr])
                elif g > 1:
                    nc.vector.tensor_add(
                        out=out_sb[:, g], in0=out_sb[:, g], in1=psA[:, g]
                    )
                    nc.scalar.dma_start(out=o[:, g], in_=out_sb[:, g])

    nc.finalize()
    return nc


_NC_CACHE = None


def _get_nc() -> bass.Bass:
    global _NC_CACHE
    if _NC_CACHE is None:
        _NC_CACHE = _build_nc()
    return _NC_CACHE


def _in_maps(h, x, a_diag, p_vec, q_vec, b_mat):
    bmax = float(np.abs(b_mat).max())
    S = E3M4_MAX_SAFE / bmax if bmax > 0 else 1.0

    # Replicated inputs, k-on-partitions chunk layout.
    # xq[ki, ko, b] = x[b, ko*128 + ki] / S   (b descale folded into x)
    xq = np.ascontiguousarray(
        (x / S).astype(NP_F16).reshape(B, KO, P).transpose(2, 1, 0)
    )
    ht = np.ascontiguousarray(
        (h * HT_SCALE).astype(NP_F8).reshape(B, KO, P).transpose(2, 1, 0)
    )
    qk = np.ascontiguousarray(
        (q_vec * QK_SCALE).astype(NP_F8).reshape(KO, P, R).transpose(1, 0, 2)
    )

    # bq[ko, ki, c, g, j] = S*b[ko*128+ki, (c*8+g)*128+j], quantized once.
    bq = (b_mat * S).astype(NP_F8).reshape(KO, P, NCORES, G, P)

    in_maps = []
    for c in range(NCORES):
        j0 = c * JS
        bc = np.ascontiguousarray(bq[:, :, c].transpose(2, 1, 0, 3))  # (G,P,KO,P)
        # hd[j, g, b] = h[b, j0 + g*128 + j];  ad[j, g] = a_diag[j0 + g*128 + j]
        hdc = np.ascontiguousarray(
            h[:, j0 : j0 + JS].astype(NP_F16).reshape(B, G, P).transpose(2, 1, 0)
        )
        adc = np.ascontiguousarray(
            a_diag[j0 : j0 + JS].reshape(G, P).T.astype(np.float32)
        )
        in_maps.append(
            {
                "xq": xq,
                "ht": ht,
                "qk": qk,
                "pt": np.ascontiguousarray(p_vec[j0 : j0 + JS, :].T.astype(NP_F16)),
                "bm": bc,
                "hd": hdc,
                "ad": adc,
            }
        )
    return in_maps


def kernel(h, x, a_diag, p_vec, q_vec, b_mat) -> np.ndarray:
    h = np.ascontiguousarray(np.asarray(h, dtype=np.float32))
    x = np.ascontiguousarray(np.asarray(x, dtype=np.float32))
    a_diag = np.asarray(a_diag, dtype=np.float32)
    p_vec = np.asarray(p_vec, dtype=np.float32)
    q_vec = np.asarray(q_vec, dtype=np.float32)
    b_mat = np.asarray(b_mat, dtype=np.float32)

    nc = _get_nc()
    res = run_bass_kernel_spmd(
        nc, _in_maps(h, x, a_diag, p_vec, q_vec, b_mat), core_ids=list(range(NCORES))
    )
    # o[j, g, b] -> out[:, c*1024 + g*128 + j]
    outs = [
        np.asarray(r["o"]).astype(np.float32).transpose(2, 1, 0).reshape(B, JS)
        for r in res.results
    ]
    return np.concatenate(outs, axis=1)


# revision 8
# speedup vs baseline: 3.3889x; 1.0755x over previous
"""DPLR SSM block kernel for Trainium2, 8 NeuronCores.

Math:  out = h @ (diag(a_diag) + p q^T).T + x @ b_mat          (B=64, H=8192, R=4)
           = h * a_diag  +  (h @ q) @ p^T  +  x @ b_mat

The dense (H,H) DPLR matrix is never materialized.  Sharding: b_mat columns
(= output features) split 8 ways; core c computes out[:, c*1024:(c+1)*1024].

HBM-bound on streaming b_mat; the correctness budget (rel_err < 2e-2) is
spent on quantization:
  - b as fp8 e3m4 (1 byte/elem), global scale S with max|S*b| ~ 15.49 (just
    under e3m4 max finite).  The descale is folded into x on the host
    (xq = x/S as fp16), so the device never rescales.
  - h/q feeding the tiny rank-4 term ride as e3m4 too (2h and 512q; the
    1/1024 descale folds into the one hq PSUM->SBUF copy).
  - output stored as fp16, upcast on host.
Measured end-to-end rel_fro error: ~1.21e-2 (gate 2e-2).

Per-core HBM traffic: b 8 MB + x 1 MB fp16 + h 0.5 MB fp8 + ~0.2 MB small
aux + 0.125 MB out ~ 9.7 MB (~27 us at the 360 GB/s modeled DMA roofline),
vs 36.6 MB for the baseline split-bf16 kernel.

PE layout: batch (64) on the moving operand, b column-blocks (128 wide)
stationary -> 64 moving rows per (chunk, group) matmul: ~16 us PE, under the
DMA roofline.  b streams GROUP-MAJOR (all 64 k-chunks of a 128-column group
consecutively), so group g's PSUM accumulation finishes at (g+1)/8 of the
stream and its epilogue + 16 KB output DMA ride mid-stream; only the last
group's epilogue is on the tail.  The rank-4 term goes through a second PSUM
bank and is pre-folded with the diagonal term into out_sb mid-stream:
    fold (mid-stream):  out_sb[:,g,:] = hd_g * ad_g + psR_g     [DVE, fused]
    epilogue per group: out_sb[:,g,:] += psA_g ; DMA out        [DVE ring]

Per core c (j0 = c*1024, groups g of 128 columns):
  psA[:,g,:] (128,64) = sum_ko  bS[g,ko]^T(128x128) . xq[ko](128x64) [PE f8xf16]
  pshq (4,64)         = sum_ko  qk[ko]^T(128x4) . ht[ko](128x64)     [PE f8]
  hqt (4,64)          = pshq / 1024                                  [DVE]
  psR[:,g,:] (128,64) = pt[g]^T(4x128) . hqt(4x64)                   [PE f16]
"""

import ml_dtypes
import numpy as np

import concourse.bass as bass
import concourse.mybir as mybir
from concourse import bacc
from concourse.bass_utils import run_bass_kernel_spmd
from concourse.tile import TileContext

H = 8192
R = 4
B = 64
NCORES = 8
JS = H // NCORES  # 1024 output columns per core
P = 128
G = JS // P  # 8 column groups of 128 per core
KO = H // P  # 64 k-chunks

F32 = mybir.dt.float32
F16 = mybir.dt.float16
F8 = mybir.dt.float8e3

NP_F16 = np.float16
NP_F8 = ml_dtypes.float8_e3m4
E3M4_MAX_SAFE = 15.49  # just under e3m4 max finite (15.5); no overflow to inf
HT_SCALE = 2.0  # pre-scale on h for the hq matmul (fewer e3m4 subnormals)
QK_SCALE = 512.0  # pre-scale on q (e3m4 subnormal floor is 2^-6)
PT_SCALE = 512.0  # pre-scale on p for its e3m4 carry

# b-tile taper in k-chunks per group: big tiles for full-rate streaming,
# small tail tiles on the LAST group so the final-byte -> final-matmul ->
# epilogue chain is short.
TILES_MID = [32, 32]
TILES_LAST = [32, 16, 8, 4, 4]
MAXKT = 32


def _build_nc(bufs: int = 12) -> bass.Bass:
    nc = bacc.Bacc("TRN2", target_bir_lowering=False, debug=False, num_devices=NCORES)

    xq = nc.dram_tensor("xq", (P, KO, B), F16, kind="ExternalInput")
    ht = nc.dram_tensor("ht", (P, KO, B), F8, kind="ExternalInput")
    qk = nc.dram_tensor("qk", (P, KO, R), F8, kind="ExternalInput")
    pt = nc.dram_tensor("pt", (R, JS), F8, kind="ExternalInput")
    bm = nc.dram_tensor("bm", (G, P, KO, P), F8, kind="ExternalInput")
    hd = nc.dram_tensor("hd", (P, G, B), F8, kind="ExternalInput")
    ad = nc.dram_tensor("ad", (P, G), F32, kind="ExternalInput")
    o = nc.dram_tensor("o", (P, G, B), F16, kind="ExternalOutput")

    with TileContext(nc) as tc:
        with (
            tc.tile_pool(name="persist", bufs=1) as persist,
            tc.tile_pool(name="bpool", bufs=bufs) as bpool,
            tc.tile_pool(name="psum", bufs=1, space="PSUM") as psum_pool,
        ):
            xq_sb = persist.tile([P, KO, B], F16)
            ht_sb = persist.tile([P, KO, B], F8)
            qk_sb = persist.tile([P, KO, R], F8)
            pt_sb = persist.tile([R, JS], F8)
            hd_sb = persist.tile([P, G, B], F8)
            ad_sb = persist.tile([P, G], F32)
            out_sb = persist.tile([P, G, B], F16)
            hqt_sb = persist.tile([R, B], F16)

            psA = psum_pool.tile([P, G, B], F32)  # main accum, one PSUM bank
            psR = psum_pool.tile([P, G, B], F32)  # rank-4 term, one PSUM bank
            pshq = psum_pool.tile([R, B], F32)

            # Aux stream on the Activation HWDGE ring.  xq first (the first
            # main matmuls need it), everything else behind it; b tiles ride
            # the SP ring concurrently and interleave at the DMA engines.
            nc.scalar.dma_start(out=xq_sb[:, 0:MAXKT], in_=xq[:, 0:MAXKT])
            nc.scalar.dma_start(out=xq_sb[:, MAXKT:KO], in_=xq[:, MAXKT:KO])
            nc.scalar.dma_start(out=ht_sb[:], in_=ht[:, :, :])
            nc.scalar.dma_start(out=qk_sb[:], in_=qk[:, :, :])
            nc.scalar.dma_start(out=hd_sb[:], in_=hd[:, :, :])
            nc.scalar.dma_start(out=ad_sb[:], in_=ad[:, :])
            nc.scalar.dma_start(out=pt_sb[:], in_=pt[:, :])

            n_main = KO * G
            im = 0
            for g in range(G):
                tiles = TILES_LAST if g == G - 1 else TILES_MID
                ko = 0
                for kt in tiles:
                    bfull = bpool.tile([P, MAXKT, P], F8, name="btile")
                    btile = bfull[:, :kt]
                    nc.sync.dma_start(out=btile[:], in_=bm[g, :, ko : ko + kt])
                    for k4 in range(kt):
                        nc.tensor.matmul(
                            psA[:, g],
                            btile[:, k4],
                            xq_sb[:, ko],
                            start=(im == 0),
                            stop=(im == n_main - 1),
                        )
                        im += 1
                        ko += 1

                if g == 1:
                    # After two groups of mains the PE has plenty of slack
                    # and ht/qk/pt have landed: fold the whole DPLR low-rank
                    # path here, mid-stream.
                    for ko_q in range(KO):
                        nc.tensor.matmul(
                            pshq[:],
                            qk_sb[:, ko_q],
                            ht_sb[:, ko_q],
                            start=(ko_q == 0),
                            stop=(ko_q == KO - 1),
                        )
                    nc.vector.tensor_scalar_mul(
                        hqt_sb[:], pshq[:], 1.0 / (HT_SCALE * QK_SCALE * PT_SCALE)
                    )
                    for gr in range(G):
                        nc.tensor.matmul(
                            psR[:, gr],
                            pt_sb[:, gr * P : (gr + 1) * P],
                            hqt_sb[:],
                            start=(gr == 0),
                            stop=(gr == G - 1),
                        )
                    for gr in range(G):
                        nc.vector.scalar_tensor_tensor(
                            out=out_sb[:, gr],
                            in0=hd_sb[:, gr],
                            scalar=ad_sb[:, gr : gr + 1],
                            in1=psR[:, gr],
                            op0=mybir.AluOpType.mult,
                            op1=mybir.AluOpType.add,
                        )
                    # groups 0 and 1 finished their mains before the fold:
                    # emit their epilogue adds now.
                    for gr in (0, 1):
                        nc.vector.tensor_add(
                            out=out_sb[:, gr], in0=out_sb[:, gr], in1=psA[:, gr]
                        )
                elif g > 1:
                    nc.vector.tensor_add(
                        out=out_sb[:, g], in0=out_sb[:, g], in1=psA[:, g]
                    )
                # Outputs ride in 3 batched DMAs so the tail b-tiles' DMA
                # lane predecessors stay old (no completion-wait chains).
                if g == 3:
                    nc.scalar.dma_start(out=o[:, 0:4], in_=out_sb[:, 0:4])
                elif g == G - 2:
                    nc.scalar.dma_start(out=o[:, 4:7], in_=out_sb[:, 4:7])
                elif g == G - 1:
                    nc.sync.dma_start(out=o[:, 7], in_=out_sb[:, 7])

    nc.finalize()
    return nc


_NC_CACHE = None


def _get_nc() -> bass.Bass:
    global _NC_CACHE
    if _NC_CACHE is None:
        _NC_CACHE = _build_nc()
    return _NC_CACHE


def _in_maps(h, x, a_diag, p_vec, q_vec, b_mat):
    bmax = float(np.abs(b_mat).max())
    S = E3M4_MAX_SAFE / bmax if bmax > 0 else 1.0

    # Replicated inputs, k-on-partitions chunk layout.
    # xq[ki, ko, b] = x[b, ko*128 + ki] / S   (b descale folded into x)
    xq = np.ascontiguousarray(
        (x / S).astype(NP_F16).reshape(B, KO, P).transpose(2, 1, 0)
    )
    ht = np.ascontiguousarray(
        (h * HT_SCALE).astype(NP_F8).reshape(B, KO, P).transpose(2, 1, 0)
    )
    qk = np.ascontiguousarray(
        (q_vec * QK_SCALE).astype(NP_F8).reshape(KO, P, R).transpose(1, 0, 2)
    )

    # bq[ko, ki, c, g, j] = S*b[ko*128+ki, (c*8+g)*128+j], quantized once.
    bq = (b_mat * S).astype(NP_F8).reshape(KO, P, NCORES, G, P)

    in_maps = []
    for c in range(NCORES):
        j0 = c * JS
        bc = np.ascontiguousarray(bq[:, :, c].transpose(2, 1, 0, 3))  # (G,P,KO,P)
        # hd[j, g, b] = h[b, j0 + g*128 + j];  ad[j, g] = a_diag[j0 + g*128 + j]
        hdc = np.ascontiguousarray(
            h[:, j0 : j0 + JS].astype(NP_F8).reshape(B, G, P).transpose(2, 1, 0)
        )
        adc = np.ascontiguousarray(
            a_diag[j0 : j0 + JS].reshape(G, P).T.astype(np.float32)
        )
        in_maps.append(
            {
                "xq": xq,
                "ht": ht,
                "qk": qk,
                "pt": np.ascontiguousarray((p_vec[j0 : j0 + JS, :] * PT_SCALE).T.astype(NP_F8)),
                "bm": bc,
                "hd": hdc,
                "ad": adc,
            }
        )
    return in_maps


def kernel(h, x, a_diag, p_vec, q_vec, b_mat) -> np.ndarray:
    h = np.ascontiguousarray(np.asarray(h, dtype=np.float32))
    x = np.ascontiguousarray(np.asarray(x, dtype=np.float32))
    a_diag = np.asarray(a_diag, dtype=np.float32)
    p_vec = np.asarray(p_vec, dtype=np.float32)
    q_vec = np.asarray(q_vec, dtype=np.float32)
    b_mat = np.asarray(b_mat, dtype=np.float32)

    nc = _get_nc()
    res = run_bass_kernel_spmd(
        nc, _in_maps(h, x, a_diag, p_vec, q_vec, b_mat), core_ids=list(range(NCORES))
    )
    # o[j, g, b] -> out[:, c*1024 + g*128 + j]
    outs = [
        np.asarray(r["o"]).astype(np.float32).transpose(2, 1, 0).reshape(B, JS)
        for r in res.results
    ]
    return np.concatenate(outs, axis=1)


# revision 11
# speedup vs baseline: 3.3928x; 1.0012x over previous
"""DPLR SSM block kernel for Trainium2, 8 NeuronCores.

Math:  out = h @ (diag(a_diag) + p q^T).T + x @ b_mat          (B=64, H=8192, R=4)
           = h * a_diag  +  (h @ q) @ p^T  +  x @ b_mat

The dense (H,H) DPLR matrix is never materialized.  Sharding: b_mat columns
(= output features) split 8 ways; core c computes out[:, c*1024:(c+1)*1024].

HBM-bound on streaming b_mat; the correctness budget (rel_err < 2e-2) is
spent on quantization:
  - b as fp8 e3m4 (1 byte/elem), global scale S with max|S*b| ~ 15.49 (just
    under e3m4 max finite).  The descale is folded into x on the host
    (xq = x/S as fp16), so the device never rescales.
  - h/q feeding the tiny rank-4 term ride as e3m4 too (2h and 512q; the
    1/1024 descale folds into the one hq PSUM->SBUF copy).
  - output stored as fp16, upcast on host.
Measured end-to-end rel_fro error: ~1.21e-2 (gate 2e-2).

Per-core HBM traffic: b 8 MB + x 1 MB fp16 + h 0.5 MB fp8 + ~0.2 MB small
aux + 0.125 MB out ~ 9.7 MB (~27 us at the 360 GB/s modeled DMA roofline),
vs 36.6 MB for the baseline split-bf16 kernel.

PE layout: batch (64) on the moving operand, b column-blocks (128 wide)
stationary -> 64 moving rows per (chunk, group) matmul: ~16 us PE, under the
DMA roofline.  b streams GROUP-MAJOR (all 64 k-chunks of a 128-column group
consecutively), so group g's PSUM accumulation finishes at (g+1)/8 of the
stream and its epilogue + 16 KB output DMA ride mid-stream; only the last
group's epilogue is on the tail.  The rank-4 term goes through a second PSUM
bank and is pre-folded with the diagonal term into out_sb mid-stream:
    fold (mid-stream):  out_sb[:,g,:] = hd_g * ad_g + psR_g     [DVE, fused]
    epilogue per group: out_sb[:,g,:] += psA_g ; DMA out        [DVE ring]

Per core c (j0 = c*1024, groups g of 128 columns):
  psA[:,g,:] (128,64) = sum_ko  bS[g,ko]^T(128x128) . xq[ko](128x64) [PE f8xf16]
  pshq (4,64)         = sum_ko  qk[ko]^T(128x4) . ht[ko](128x64)     [PE f8]
  hqt (4,64)          = pshq / 1024                                  [DVE]
  psR[:,g,:] (128,64) = pt[g]^T(4x128) . hqt(4x64)                   [PE f16]
"""

import ml_dtypes
import numpy as np

import concourse.bass as bass
import concourse.mybir as mybir
from concourse import bacc
from concourse.bass_utils import run_bass_kernel_spmd
from concourse.tile import TileContext

H = 8192
R = 4
B = 64
NCORES = 8
JS = H // NCORES  # 1024 output columns per core
P = 128
G = JS // P  # 8 column groups of 128 per core
KO = H // P  # 64 k-chunks

F32 = mybir.dt.float32
F16 = mybir.dt.float16
F8 = mybir.dt.float8e3

NP_F16 = np.float16
NP_F8 = ml_dtypes.float8_e3m4
E3M4_MAX_SAFE = 15.49  # just under e3m4 max finite (15.5); no overflow to inf
HT_SCALE = 2.0  # pre-scale on h for the hq matmul (fewer e3m4 subnormals)
QK_SCALE = 512.0  # pre-scale on q (e3m4 subnormal floor is 2^-6)
PT_SCALE = 512.0  # pre-scale on p for its e3m4 carry

# b-tile taper in k-chunks per group: big tiles for full-rate streaming,
# small tail tiles on the LAST group so the final-byte -> final-matmul ->
# epilogue chain is short.
TILES_MID = [32, 32]
TILES_LAST = [32, 18, 8, 4, 1, 1]
MAXKT = 32


def _build_nc(bufs: int = 12) -> bass.Bass:
    nc = bacc.Bacc("TRN2", target_bir_lowering=False, debug=False, num_devices=NCORES)

    xq = nc.dram_tensor("xq", (P, KO, B), F16, kind="ExternalInput")
    ht = nc.dram_tensor("ht", (P, KO, B), F8, kind="ExternalInput")
    qk = nc.dram_tensor("qk", (P, KO, R), F8, kind="ExternalInput")
    pt = nc.dram_tensor("pt", (R, JS), F8, kind="ExternalInput")
    bm = nc.dram_tensor("bm", (G, P, KO, P), F8, kind="ExternalInput")
    hd = nc.dram_tensor("hd", (P, G, B), F8, kind="ExternalInput")
    ad = nc.dram_tensor("ad", (P, G), F32, kind="ExternalInput")
    o = nc.dram_tensor("o", (P, G, B), F16, kind="ExternalOutput")

    with TileContext(nc) as tc:
        with (
            tc.tile_pool(name="persist", bufs=1) as persist,
            tc.tile_pool(name="bpool", bufs=bufs) as bpool,
            tc.tile_pool(name="psum", bufs=1, space="PSUM") as psum_pool,
        ):
            xq_sb = persist.tile([P, KO, B], F16)
            ht_sb = persist.tile([P, KO, B], F8)
            qk_sb = persist.tile([P, KO, R], F8)
            pt_sb = persist.tile([R, JS], F8)
            hd_sb = persist.tile([P, G, B], F8)
            ad_sb = persist.tile([P, G], F32)
            out_sb = persist.tile([P, G, B], F16)
            hqt_sb = persist.tile([R, B], F16)

            psA = psum_pool.tile([P, G, B], F32)  # main accum, one PSUM bank
            psR = psum_pool.tile([P, G, B], F32)  # rank-4 term, one PSUM bank
            pshq = psum_pool.tile([R, B], F32)

            # Aux stream on the Activation HWDGE ring.  xq first (the first
            # main matmuls need it), everything else behind it; b tiles ride
            # the SP ring concurrently and interleave at the DMA engines.
            nc.scalar.dma_start(out=xq_sb[:, 0:MAXKT], in_=xq[:, 0:MAXKT])
            nc.scalar.dma_start(out=xq_sb[:, MAXKT:KO], in_=xq[:, MAXKT:KO])
            nc.scalar.dma_start(out=ht_sb[:], in_=ht[:, :, :])
            nc.scalar.dma_start(out=qk_sb[:], in_=qk[:, :, :])
            nc.scalar.dma_start(out=hd_sb[:], in_=hd[:, :, :])
            nc.scalar.dma_start(out=ad_sb[:], in_=ad[:, :])
            nc.scalar.dma_start(out=pt_sb[:], in_=pt[:, :])

            n_main = KO * G
            im = 0
            for g in range(G):
                tiles = TILES_LAST if g == G - 1 else TILES_MID
                ko = 0
                for kt in tiles:
                    bfull = bpool.tile([P, MAXKT, P], F8, name="btile")
                    btile = bfull[:, :kt]
                    nc.sync.dma_start(out=btile[:], in_=bm[g, :, ko : ko + kt])
                    for k4 in range(kt):
                        nc.tensor.matmul(
                            psA[:, g],
                            btile[:, k4],
                            xq_sb[:, ko],
                            start=(im == 0),
                            stop=(im == n_main - 1),
                        )
                        im += 1
                        ko += 1

                if g == 1:
                    # After two groups of mains the PE has plenty of slack
                    # and ht/qk/pt have landed: fold the whole DPLR low-rank
                    # path here, mid-stream.
                    for ko_q in range(KO):
                        nc.tensor.matmul(
                            pshq[:],
                            qk_sb[:, ko_q],
                            ht_sb[:, ko_q],
                            start=(ko_q == 0),
                            stop=(ko_q == KO - 1),
                        )
                    nc.vector.tensor_scalar_mul(
                        hqt_sb[:], pshq[:], 1.0 / (HT_SCALE * QK_SCALE * PT_SCALE)
                    )
                    for gr in range(G):
                        nc.tensor.matmul(
                            psR[:, gr],
                            pt_sb[:, gr * P : (gr + 1) * P],
                            hqt_sb[:],
                            start=(gr == 0),
                            stop=(gr == G - 1),
                        )
                    for gr in range(G):
                        nc.vector.scalar_tensor_tensor(
                            out=out_sb[:, gr],
                            in0=hd_sb[:, gr],
                            scalar=ad_sb[:, gr : gr + 1],
                            in1=psR[:, gr],
                            op0=mybir.AluOpType.mult,
                            op1=mybir.AluOpType.add,
                        )
                    # groups 0 and 1 finished their mains before the fold:
                    # emit their epilogue adds now.
                    for gr in (0, 1):
                        nc.vector.tensor_add(
                            out=out_sb[:, gr], in0=out_sb[:, gr], in1=psA[:, gr]
                        )
                elif g > 1:
                    nc.vector.tensor_add(
                        out=out_sb[:, g], in0=out_sb[:, g], in1=psA[:, g]
                    )
                # Outputs ride in 3 batched DMAs so the tail b-tiles' DMA
                # lane predecessors stay old (no completion-wait chains).
                if g == 3:
                    nc.scalar.dma_start(out=o[:, 0:4], in_=out_sb[:, 0:4])
                elif g == G - 2:
                    nc.scalar.dma_start(out=o[:, 4:7], in_=out_sb[:, 4:7])
                elif g == G - 1:
                    nc.sync.dma_start(out=o[:, 7], in_=out_sb[:, 7])

    nc.finalize()
    return nc


_NC_CACHE = None


def _get_nc() -> bass.Bass:
    global _NC_CACHE
    if _NC_CACHE is None:
        _NC_CACHE = _build_nc()
    return _NC_CACHE


def _in_maps(h, x, a_diag, p_vec, q_vec, b_mat):
    bmax = float(np.abs(b_mat).max())
    S = E3M4_MAX_SAFE / bmax if bmax > 0 else 1.0

    # Replicated inputs, k-on-partitions chunk layout.
    # xq[ki, ko, b] = x[b, ko*128 + ki] / S   (b descale folded into x)
    xq = np.ascontiguousarray(
        (x / S).astype(NP_F16).reshape(B, KO, P).transpose(2, 1, 0)
    )
    ht = np.ascontiguousarray(
        (h * HT_SCALE).astype(NP_F8).reshape(B, KO, P).transpose(2, 1, 0)
    )
    qk = np.ascontiguousarray(
        (q_vec * QK_SCALE).astype(NP_F8).reshape(KO, P, R).transpose(1, 0, 2)
    )

    # bq[ko, ki, c, g, j] = S*b[ko*128+ki, (c*8+g)*128+j], quantized once.
    bq = (b_mat * S).astype(NP_F8).reshape(KO, P, NCORES, G, P)

    in_maps = []
    for c in range(NCORES):
        j0 = c * JS
        bc = np.ascontiguousarray(bq[:, :, c].transpose(2, 1, 0, 3))  # (G,P,KO,P)
        # hd[j, g, b] = h[b, j0 + g*128 + j];  ad[j, g] = a_diag[j0 + g*128 + j]
        hdc = np.ascontiguousarray(
            h[:, j0 : j0 + JS].astype(NP_F8).reshape(B, G, P).transpose(2, 1, 0)
        )
        adc = np.ascontiguousarray(
            a_diag[j0 : j0 + JS].reshape(G, P).T.astype(np.float32)
        )
        in_maps.append(
            {
                "xq": xq,
                "ht": ht,
                "qk": qk,
                "pt": np.ascontiguousarray((p_vec[j0 : j0 + JS, :] * PT_SCALE).T.astype(NP_F8)),
                "bm": bc,
                "hd": hdc,
                "ad": adc,
            }
        )
    return in_maps


def kernel(h, x, a_diag, p_vec, q_vec, b_mat) -> np.ndarray:
    h = np.ascontiguousarray(np.asarray(h, dtype=np.float32))
    x = np.ascontiguousarray(np.asarray(x, dtype=np.float32))
    a_diag = np.asarray(a_diag, dtype=np.float32)
    p_vec = np.asarray(p_vec, dtype=np.float32)
    q_vec = np.asarray(q_vec, dtype=np.float32)
    b_mat = np.asarray(b_mat, dtype=np.float32)

    nc = _get_nc()
    res = run_bass_kernel_spmd(
        nc, _in_maps(h, x, a_diag, p_vec, q_vec, b_mat), core_ids=list(range(NCORES))
    )
    # o[j, g, b] -> out[:, c*1024 + g*128 + j]
    outs = [
        np.asarray(r["o"]).astype(np.float32).transpose(2, 1, 0).reshape(B, JS)
        for r in res.results
    ]
    return np.concatenate(outs, axis=1)


# revision 17
# speedup vs baseline: 3.4822x; 1.0264x over previous
"""DPLR SSM block kernel for Trainium2, 8 NeuronCores.

Math:  out = h @ (diag(a_diag) + p q^T).T + x @ b_mat          (B=64, H=8192, R=4)
           = h * a_diag  +  (h @ q) @ p^T  +  x @ b_mat

The dense (H,H) DPLR matrix is never materialized.  Sharding: b_mat columns
(= output features) split 8 ways; core c computes out[:, c*1024:(c+1)*1024].

HBM-bound on streaming b_mat; the correctness budget (rel_err < 2e-2) is
spent on quantization:
  - b as fp8 e3m4 (1 byte/elem), global scale S with max|S*b| ~ 15.49 (just
    under e3m4 max finite).  The descale is folded into x on the host
    (xq = x/S as fp16), so the device never rescales.
  - h/q/p feeding the tiny rank-4 + diagonal terms ride as e3m4 too, with
    pre-scales whose product folds into the one hq PSUM->SBUF copy.
  - output stored as fp16, upcast on host.
Measured end-to-end rel_fro error: ~1.21e-2 (gate 2e-2).

Per-core HBM traffic ~9.7 MB (vs 36.6 MB for the baseline split-bf16
kernel): b 8 MB fp8 + x 1 MB fp16 + h 0.5 MB fp8 + ~0.2 MB small aux +
0.125 MB out.  The modeled DMA roofline (360 GB/s, all queues serialized
through one DMA-engine pool) makes this a ~27 us stream; PE work is ~16 us
and hides under it.

PE layout: batch (64) on the moving operand, b column-blocks (128 wide)
stationary -> 64 moving rows per (chunk, group) matmul.  b streams
GROUP-MAJOR (all 64 k-chunks of a 128-column group consecutively), so group
g's PSUM accumulation finishes at (g+1)/8 of the stream; its epilogue and
output ride mid-stream and only the last group's epilogue is on the tail.
The rank-4 term goes through a second PSUM bank and is pre-folded with the
diagonal term into out_sb mid-stream:
    fold (mid-stream):  out_sb[g] = hd_g * ad_g + psR_g      [DVE, fused]
    epilogue per group: out_sb[g] += psA_g                   [DVE]

Tail: the final two groups' 32 KB store goes through a SWDGE
prepare/trigger pair -- descriptors are written mid-stream on the idle
GPSIMD queue and the tail pays a ~40ns trigger + transfer instead of the
~1.3us HWDGE issue chain of a regular dma_start.  The scatter-add target
region of `o` is zeroed by a small DMA up front.

Per core c (j0 = c*1024, groups g of 128 columns):
  psA[:,g,:] (128,64) = sum_ko  bS[g,ko]^T(128x128) . xq[ko](128x64) [PE f8xf16]
  pshq (4,64)         = sum_ko  qk[ko]^T(128x4) . ht[ko](128x64)     [PE f8]
  hqt (4,64)          = pshq / (HT*QK*PT scales)                     [DVE]
  psR[:,g,:] (128,64) = pt[g]^T(4x128) . hqt(4x64)                   [PE f16]
"""

import ml_dtypes
import numpy as np

import concourse.bass as bass
import concourse.mybir as mybir
from concourse import bacc
from concourse.bass_utils import run_bass_kernel_spmd
from concourse.tile import TileContext

H = 8192
R = 4
B = 64
NCORES = 8
JS = H // NCORES  # 1024 output columns per core
P = 128
G = JS // P  # 8 column groups of 128 per core
GP = G // 2  # groups pair up in the output layout (256B scatter rows)
KO = H // P  # 64 k-chunks

F32 = mybir.dt.float32
F16 = mybir.dt.float16
F8 = mybir.dt.float8e3
I16 = mybir.dt.int16

NP_F16 = np.float16
NP_F8 = ml_dtypes.float8_e3m4
E3M4_MAX_SAFE = 15.49  # just under e3m4 max finite (15.5); no overflow to inf
HT_SCALE = 2.0  # pre-scale on h for the hq matmul (fewer e3m4 subnormals)
QK_SCALE = 512.0  # pre-scale on q (e3m4 subnormal floor is 2^-6)
PT_SCALE = 512.0  # pre-scale on p for its e3m4 carry

# b-tile taper in k-chunks per group: big tiles for full-rate streaming,
# small tail tiles on the LAST group so the final-byte -> final-matmul ->
# epilogue chain is short.
TILES_MID = [32, 32]
TILES_LAST = [32, 20, 8, 4]
MAXKT = 32


def _build_nc(bufs: int = 12) -> bass.Bass:
    nc = bacc.Bacc("TRN2", target_bir_lowering=False, debug=False, num_devices=NCORES)

    xq = nc.dram_tensor("xq", (P, KO, B), F16, kind="ExternalInput")
    hq8 = nc.dram_tensor("hq8", (P, KO, B + R), F8, kind="ExternalInput")
    pt = nc.dram_tensor("pt", (R, JS), F8, kind="ExternalInput")
    bm = nc.dram_tensor("bm", (G, P, KO, P), F8, kind="ExternalInput")
    hd = nc.dram_tensor("hd", (P, G, B), F8, kind="ExternalInput")
    ad = nc.dram_tensor("ad", (P, G), F32, kind="ExternalInput")
    ix = nc.dram_tensor("ix", (P, G), I16, kind="ExternalInput")
    o = nc.dram_tensor("o", (P, GP, 2 * B), F16, kind="ExternalOutput")

    sc_sem = nc.alloc_semaphore("sc7")

    with TileContext(nc) as tc:
        with (
            tc.tile_pool(name="persist", bufs=1) as persist,
            tc.tile_pool(name="bpool", bufs=bufs) as bpool,
            tc.tile_pool(name="psum", bufs=1, space="PSUM") as psum_pool,
        ):
            xq_sb = persist.tile([P, KO, B], F16)
            hq8_sb = persist.tile([P, KO, B + R], F8)
            pt_sb = persist.tile([R, JS], F8)
            hd_sb = persist.tile([P, G, B], F8)
            ad_sb = persist.tile([P, G], F32)
            ix_sb = persist.tile([P, G], I16)
            z_sb = persist.tile([P, 1, 2 * B], F16)
            out_sb = persist.tile([P, GP, 2 * B], F16)
            hqt_sb = persist.tile([R, B], F16)

            psA = psum_pool.tile([P, G, B], F32)  # main accum, one PSUM bank
            psR = psum_pool.tile([P, G, B], F32)  # rank-4 term, one PSUM bank
            pshq = psum_pool.tile([R, B], F32)

            def oap(g):
                # group g's [128, 64] slice of the paired output layout
                return out_sb[:, g // 2, (g % 2) * B : (g % 2 + 1) * B]

            # Aux stream on the Activation HWDGE ring.  xq first (the first
            # main matmuls need it), everything else behind it; b tiles ride
            # the SP ring concurrently and interleave at the DMA engines.
            nc.scalar.dma_start(out=xq_sb[:], in_=xq[:, :, :])
            nc.scalar.dma_start(out=hq8_sb[:], in_=hq8[:, :, :])
            nc.scalar.dma_start(out=hd_sb[:], in_=hd[:, :, :])
            nc.scalar.dma_start(out=ad_sb[:], in_=ad[:, :])
            nc.scalar.dma_start(out=pt_sb[:], in_=pt[:, :])
            nc.scalar.dma_start(out=ix_sb[:], in_=ix[:, :])
            # Zero the scatter-add target region (groups 6-7 of o).
            nc.vector.memset(z_sb[:], 0.0)
            nc.scalar.dma_start(out=o[:, GP - 1 : GP], in_=z_sb[:])

            n_main = KO * G
            im = 0
            for g in range(G):
                tiles = TILES_LAST if g == G - 1 else TILES_MID
                ko = 0
                for kt in tiles:
                    bfull = bpool.tile([P, MAXKT, P], F8, name="btile")
                    btile = bfull[:, :kt]
                    nc.sync.dma_start(out=btile[:], in_=bm[g, :, ko : ko + kt])
                    for k4 in range(kt):
                        nc.tensor.matmul(
                            psA[:, g],
                            btile[:, k4],
                            xq_sb[:, ko],
                            start=(im == 0),
                            stop=(im == n_main - 1),
                        )
                        im += 1
                        ko += 1

                if g == 1:
                    # After two groups of mains the PE has plenty of slack
                    # and ht/qk/pt have landed: fold the whole DPLR low-rank
                    # path here, mid-stream.
                    for ko_q in range(KO):
                        nc.tensor.matmul(
                            pshq[:],
                            hq8_sb[:, ko_q, B : B + R],
                            hq8_sb[:, ko_q, 0:B],
                            start=(ko_q == 0),
                            stop=(ko_q == KO - 1),
                        )
                    nc.vector.tensor_scalar_mul(
                        hqt_sb[:], pshq[:], 1.0 / (HT_SCALE * QK_SCALE * PT_SCALE)
                    )
                    for gr in range(G):
                        nc.tensor.matmul(
                            psR[:, gr],
                            pt_sb[:, gr * P : (gr + 1) * P],
                            hqt_sb[:],
                            start=(gr == 0),
                            stop=(gr == G - 1),
                        )
                    for gr in range(G):
                        nc.vector.scalar_tensor_tensor(
                            out=oap(gr),
                            in0=hd_sb[:, gr],
                            scalar=ad_sb[:, gr : gr + 1],
                            in1=psR[:, gr],
                            op0=mybir.AluOpType.mult,
                            op1=mybir.AluOpType.add,
                        )
                    # SWDGE descriptors for the final groups' store; the DMA
                    # itself fires from trigger_dma at the tail.  Data deps
                    # (the epilogue adds into out_sb[:, 3]) sit on the
                    # trigger, not the prep.
                    nc.gpsimd.dma_scatter_add(
                        o[:, GP - 1 : GP, :],
                        out_sb[:, GP - 1 : GP, :],
                        ix_sb[:],
                        P,
                        P,
                        2 * B,
                        elem_step=G * B,
                        prepare_only=True,
                        sem=sc_sem,
                    )
                    # groups 0 and 1 finished their mains before the fold:
                    # emit their epilogue adds now.
                    for gr in (0, 1):
                        nc.vector.tensor_add(
                            out=oap(gr), in0=oap(gr), in1=psA[:, gr]
                        )
                elif g > 1:
                    nc.vector.tensor_add(out=oap(g), in0=oap(g), in1=psA[:, g])
                # Outputs for groups 0-5 ride in 2 batched mid-stream DMAs;
                # groups 6-7 go through the prepared scatter at the tail.
                if g == 3:
                    nc.scalar.dma_start(out=o[:, 0:2], in_=out_sb[:, 0:2])
                elif g == G - 2:
                    nc.scalar.dma_start(out=o[:, 2:3], in_=out_sb[:, 2:3])
                elif g == G - 1:
                    nc.gpsimd.trigger_dma(count=None)

    nc.finalize()

    # Tile assigns the scatter prep a DMASW completion lane and makes the
    # exit drain wait on that lane's semaphore, but leaves the user sem in
    # the descriptor's completion slot (on_update[0]).  Retarget the
    # completion update at the lane sem the drain actually waits on (this is
    # exactly what Tile wires up for non-prepared SWDGE DMAs).
    fn = nc.m.functions[0]
    lane_wait = None
    prep = None
    for blk in list(fn.blocks):
        for inst in list(blk.instructions):
            si = inst.sync_info
            if si is None:
                continue
            for w in si.on_wait:
                if w.ant_name and w.ant_name.startswith("DMASW"):
                    lane_wait = w
            if type(inst).__name__ == "InstDMAScatterAddAnt":
                prep = inst
    assert prep is not None and lane_wait is not None
    upd = prep.sync_info.on_update[0]
    assert upd.ant_name == "sc7", upd
    upd.id = lane_wait.id
    upd.ant_name = lane_wait.ant_name
    return nc


_NC_CACHE = None


def _get_nc() -> bass.Bass:
    global _NC_CACHE
    if _NC_CACHE is None:
        _NC_CACHE = _build_nc()
    return _NC_CACHE


def _in_maps(h, x, a_diag, p_vec, q_vec, b_mat):
    bmax = float(np.abs(b_mat).max())
    S = E3M4_MAX_SAFE / bmax if bmax > 0 else 1.0

    # Replicated inputs, k-on-partitions chunk layout.
    # xq[ki, ko, b] = x[b, ko*128 + ki] / S   (b descale folded into x)
    xq = np.ascontiguousarray(
        (x / S).astype(NP_F16).reshape(B, KO, P).transpose(2, 1, 0)
    )
    hq8 = np.empty((P, KO, B + R), dtype=NP_F8)
    hq8[:, :, 0:B] = (h * HT_SCALE).astype(NP_F8).reshape(B, KO, P).transpose(2, 1, 0)
    hq8[:, :, B : B + R] = (
        (q_vec * QK_SCALE).astype(NP_F8).reshape(KO, P, R).transpose(1, 0, 2)
    )

    # Scatter identity indices, wrapped in 16 partitions and replicated
    # across the rest: idx i decodes from [i % 16, i // 16].
    ii = np.arange(P)
    ixw = np.ascontiguousarray(
        ((ii[:, None] % 16) + 16 * np.arange(G)[None, :]).astype(np.int16)
    )

    # bq[ko, ki, c, g, j] = S*b[ko*128+ki, (c*8+g)*128+j], quantized once.
    bq = (b_mat * S).astype(NP_F8).reshape(KO, P, NCORES, G, P)

    in_maps = []
    for c in range(NCORES):
        j0 = c * JS
        bc = np.ascontiguousarray(bq[:, :, c].transpose(2, 1, 0, 3))  # (G,P,KO,P)
        # hd[j, g, b] = h[b, j0 + g*128 + j];  ad[j, g] = a_diag[j0 + g*128 + j]
        hdc = np.ascontiguousarray(
            h[:, j0 : j0 + JS].astype(NP_F8).reshape(B, G, P).transpose(2, 1, 0)
        )
        adc = np.ascontiguousarray(
            a_diag[j0 : j0 + JS].reshape(G, P).T.astype(np.float32)
        )
        in_maps.append(
            {
                "ix": ixw,
                "xq": xq,
                "hq8": hq8,
                "pt": np.ascontiguousarray(
                    (p_vec[j0 : j0 + JS, :] * PT_SCALE).T.astype(NP_F8)
                ),
                "bm": bc,
                "hd": hdc,
                "ad": adc,
            }
        )
    return in_maps


def kernel(h, x, a_diag, p_vec, q_vec, b_mat) -> np.ndarray:
    h = np.ascontiguousarray(np.asarray(h, dtype=np.float32))
    x = np.ascontiguousarray(np.asarray(x, dtype=np.float32))
    a_diag = np.asarray(a_diag, dtype=np.float32)
    p_vec = np.asarray(p_vec, dtype=np.float32)
    q_vec = np.asarray(q_vec, dtype=np.float32)
    b_mat = np.asarray(b_mat, dtype=np.float32)

    nc = _get_nc()
    res = run_bass_kernel_spmd(
        nc, _in_maps(h, x, a_diag, p_vec, q_vec, b_mat), core_ids=list(range(NCORES))
    )
    # o[j, gg, gh*64 + b] -> out[b, (2*gg + gh)*128 + j]
    outs = [
        np.asarray(r["o"])
        .astype(np.float32)
        .reshape(P, GP, 2, B)
        .transpose(3, 1, 2, 0)
        .reshape(B, JS)
        for r in res.results
    ]
    return np.concatenate(outs, axis=1)
